# revision 2
# baseline (speedup 1.0000x reference)
"""Trainium2 Bass kernel for nn_ContrastiveLoss (prototype InfoNCE loss).

Strategy (data-parallel over the N=100k cell axis, 8 NeuronCores):
  - Each core gets a 12544-row shard (rows padded with label=-1 / feat=0).
  - Per 128-row tile, a one-hot [128,64] matrix is built on-chip (DVE
    is_equal against an iota constant) and a single bf16 matmul
    one_hot.T @ [feat | 1] accumulates per-class sums AND counts into
    PSUM ([64, 257], fp32 accumulation).  Features are cast f32->bf16
    in-flight by the SWDGE DMA; the loss is insensitive to this rounding
    (validated: rel err 1.6e-7, identical to pure-f32 pipeline).
  - One 8-core AllReduce of the [128, 257] packed (atac|rna) sums+counts.
  - The tiny K=64 InfoNCE is computed replicated on every core:
    normalize prototypes, PE-transpose to [D, K] layout, broadcast
    tensor_tensor outer products + ACT exp + free-axis reductions.
  - Output: scalar loss (identical on every core).
"""
import sys

sys.path.insert(0, "/opt/trn_rl_repo")

import math
import numpy as np
from contextlib import ExitStack

N, D, K = 100000, 256, 64
NCORES = 8
NTILES = 98               # tiles of 128 rows per core
NPAD = NTILES * 128       # 12544 rows per core (total 100352 >= 100000)
CH = 16                   # 128-row tiles per DMA chunk (2 MB f32 reads)
NCHUNKS = (NTILES + CH - 1) // CH   # 12 full chunks + 1 chunk of 2
TAU = 0.5
EPS = 1e-8
C_FP = 2 * K - 3          # coefficient of Fp in Fn:  sum_{j!=k}(Sa+Sr)+2(K-1)Fp
                          #   = rowsum(Sa)+rowsum(Sr) - exp(2A.A) + (2K-3)*Fp
KB = K // NCORES          # k-rows of the InfoNCE computed per core

_cache = {}


def _build(repeat_main=1, repeat_ar=1):
    import concourse.bacc as bacc
    import concourse.tile as tile
    from concourse import mybir

    f32, bf16, i32 = mybir.dt.float32, mybir.dt.bfloat16, mybir.dt.int32
    AF = mybir.ActivationFunctionType
    OP = mybir.AluOpType

    nc = bacc.Bacc(None, target_bir_lowering=False, debug=False,
                   num_devices=NCORES)

    fa = nc.dram_tensor("fa", [NPAD, D], f32, kind="ExternalInput")
    fr = nc.dram_tensor("fr", [NPAD, D], f32, kind="ExternalInput")
    la = nc.dram_tensor("la", [NPAD], i32, kind="ExternalInput")
    lr = nc.dram_tensor("lr", [NPAD], i32, kind="ExternalInput")
    # per-core selector: cols 0:KB pick this core's A-rows (0:64), cols
    # KB:2KB pick its R-rows (64:128) out of the packed [128, D] pn tile
    ksel = nc.dram_tensor("ksel", [128, 2 * KB], f32, kind="ExternalInput")
    out = nc.dram_tensor("out", [1, 1], f32, kind="ExternalOutput")

    iota_c = nc.inline_tensor(
        np.tile(np.arange(K, dtype=np.float32), (128, 1)), name="iota_c")
    ident_c = nc.inline_tensor(np.eye(128, dtype=np.float32), name="ident_c")
    ones_c = nc.inline_tensor(np.ones((128, 1), dtype=np.float32),
                              name="ones_c")

    with tile.TileContext(nc) as tc, ExitStack() as ctx:
        consts = ctx.enter_context(tc.tile_pool(name="consts", bufs=1))
        psum = ctx.enter_context(tc.tile_pool(name="psum", bufs=1,
                                              space="PSUM"))
        dram = ctx.enter_context(tc.tile_pool(name="dram", bufs=1,
                                              space="DRAM"))

        iota_sb = consts.tile([128, K], f32)
        nc.sync.dma_start(iota_sb, iota_c[:, :])
        warm = consts.tile([1, 1], f32)
        nc.vector.memset(warm, 1.0)
        nc.scalar.activation(warm, warm, AF.Exp)
        nc.scalar.activation(warm, warm, AF.Ln)
        ident_sb = consts.tile([128, 128], f32)
        nc.sync.dma_start(ident_sb, ident_c[:, :])
        ones_sb = consts.tile([128, 1], f32)
        nc.sync.dma_start(ones_sb, ones_c[:, :])

        # ---------------- main phase: segment sums + counts ----------------
        with tc.tile_pool(name="labels", bufs=1) as labels, \
             tc.tile_pool(name="oh", bufs=1) as ohp, \
             tc.tile_pool(name="feat", bufs=3) as featp:

            # label prep: [NPAD] i32 -> (cast DMA) [98,128] f32
            #   -> PE transpose -> [128, 98] f32 (labT[p, t] = label[t*128+p])
            labT = {}
            for nm, lab in (("a", la), ("r", lr)):
                lf = labels.tile([NTILES, 128], f32, name=f"lf_{nm}")
                nc.gpsimd.dma_start(
                    lf, lab[:].rearrange("(j p) -> j p", p=128))
                psl = psum.tile([128, NTILES], f32, name=f"psl_{nm}",
                                tag="psl")
                nc.tensor.transpose(psl, lf, ident_sb[:NTILES, :NTILES])
                lt = labels.tile([128, NTILES], f32, name=f"labT_{nm}")
                nc.vector.tensor_copy(lt, psl)
                labT[nm] = lt

            # one-hots for all tiles: oh[p, t, k] = (label[t*128+p] == k)
            # split [0:CH] / [CH:] so the first chunk's matmuls start early
            oh = {}
            for nm in ("a", "r"):
                o = ohp.tile([128, NTILES, K], bf16, name=f"oh_{nm}")
                for lo, hi in ((0, CH), (CH, NTILES)):
                    w = hi - lo
                    nc.vector.tensor_tensor(
                        o[:, lo:hi, :],
                        iota_sb[:, None, :].to_broadcast([128, w, K]),
                        labT[nm][:, lo:hi, None].to_broadcast([128, w, K]),
                        OP.is_equal,
                    )
                oh[nm] = o

            # Full-partition PSUM tiles so each accumulator owns its bank
            # at base_partition 0 (packing two [64,*] tiles into one bank
            # makes the second chain a col-tiled matmul, which corrupts
            # interleaved accumulation -- seen on HW).
            psA_full = psum.tile([128, D], f32)
            psR_full = psum.tile([128, D], f32)
            psA = psA_full[0:K, :]
            psR = psR_full[0:K, :]

            for rep in range(repeat_main):
                for c in range(NCHUNKS):
                    w = min(CH, NTILES - c * CH)
                    r0 = c * CH * 128
                    fts = {}
                    for nm, feat in (("a", fa), ("r", fr)):
                        ft = featp.tile([128, CH, D], bf16, name=f"ft_{nm}",
                                        tag=f"ft_{nm}")
                        nc.gpsimd.dma_start(
                            ft[:, :w, :],
                            feat[r0:r0 + w * 128, :].rearrange(
                                "(j p) e -> p j e", p=128),
                        )
                        fts[nm] = ft
                    for j in range(w):
                        t = c * CH + j
                        nc.tensor.matmul(psA, oh["a"][:, t, :],
                                         fts["a"][:, j, :],
                                         start=(t == 0),
                                         stop=(t == NTILES - 1))
                        nc.tensor.matmul(psR, oh["r"][:, t, :],
                                         fts["r"][:, j, :],
                                         start=(t == 0),
                                         stop=(t == NTILES - 1))

            comb = consts.tile([128, D], f32)
            nc.vector.tensor_copy(comb[0:K, :], psA)
            nc.vector.tensor_copy(comb[K:128, :], psR)

        # ---------------- AllReduce sums+counts across the 8 cores ---------
        d_in = dram.tile([128, D], f32)
        d_out = dram.tile([128, D], f32)
        nc.sync.dma_start(d_in, comb)
        for _rep in range(repeat_ar):
            nc.gpsimd.collective_compute(
                "AllReduce", mybir.AluOpType.add,
                replica_groups=[list(range(NCORES))],
                ins=[d_in.opt()], outs=[d_out.opt()],
            )

        # ------- tiny K x K x D InfoNCE (k-sharded across the 8 cores) -----
        with tc.tile_pool(name="fin", bufs=1) as fin, \
             tc.tile_pool(name="pst", bufs=1, space="PSUM") as pstp:
            allr = fin.tile([128, D], f32)
            nc.sync.dma_start(allr, d_out)
            ksel_sb = fin.tile([128, 2 * KB], f32)
            nc.sync.dma_start(ksel_sb, ksel[:, :])

            # l2norm(sums/counts) == sums/||sums||: counts cancel, so we
            # never materialize them.  rinv = exp(-0.5*ln(sum(s^2)))
            sq = fin.tile([128, D], f32)
            ss = fin.tile([128, 1], f32)
            nc.scalar.activation(sq, allr, AF.Square, accum_out=ss)
            lnss = fin.tile([128, 1], f32)
            nc.scalar.activation(lnss, ss, AF.Ln)
            rinv = fin.tile([128, 1], f32)
            nc.scalar.activation(rinv, lnss, AF.Exp, scale=-0.5)

            # Fold normalization into the PE transpose:
            #   pT_h[d, i] = sum_p allr[p, h*128+d] * (ident*rinv)[p, i]
            #              = allr[i, h*128+d] * rinv[i]  (normalized, transposed)
            #   bT_h[d, s] = same with (ksel*rinv) -> this core's k-block
            dscale = fin.tile([128, 128], f32)
            nc.vector.tensor_scalar_mul(dscale, ident_sb, rinv)
            kscale = fin.tile([128, 2 * KB], f32)
            nc.vector.tensor_scalar_mul(kscale, ksel_sb, rinv)

            pT = []
            bT = []
            for h in range(2):
                half = allr[:, h * 128:(h + 1) * 128]
                pst = pstp.tile([128, 128], f32, name=f"pst_{h}", tag="pst")
                nc.tensor.matmul(pst, half, dscale, start=True, stop=True)
                sb = fin.tile([128, 128], f32, name=f"pT_{h}")
                nc.vector.tensor_copy(sb, pst)
                pT.append(sb)
                pstb = pstp.tile([128, 2 * KB], f32, name=f"pstb_{h}",
                                 tag="pstb")
                nc.tensor.matmul(pstb, half, kscale, start=True, stop=True)
                sbb = fin.tile([128, 2 * KB], f32, name=f"bT_{h}")
                nc.vector.tensor_copy(sbb, pstb)
                bT.append(sbb)

            bias_lnc = fin.tile([128, 1], f32)
            nc.vector.memset(bias_lnc, math.log(C_FP))
            bias_eps = fin.tile([128, 1], f32)
            nc.vector.memset(bias_eps, EPS)

            total = fin.tile([128, 1], f32)
            # pass 1: all DVE products + ACT Exps (one exp-set load);
            # pass 2: Fn combines + ACT Lns (one ln-set load)
            sums = {}
            for h in range(2):
                A_T = pT[h][:, 0:K]         # [128, 64] all A rows (j axis)
                R_T = pT[h][:, K:128]       # [128, 64] all R rows (j axis)
                Ab = bT[h][:, 0:KB]         # [128, 8] this core's A rows
                Rb = bT[h][:, KB:2 * KB]    # [128, 8] this core's R rows
                PA = fin.tile([128, KB, K], f32, name=f"PA_{h}", tag="PA")
                nc.vector.tensor_tensor(
                    PA,
                    Ab[:, :, None].to_broadcast([128, KB, K]),
                    A_T[:, None, :].to_broadcast([128, KB, K]),
                    OP.mult)
                SA = fin.tile([128, KB, K], f32, name=f"SA_{h}", tag="SA")
                nc.scalar.activation(SA, PA, AF.Exp, scale=1.0 / TAU)
                sumSa = fin.tile([128, KB], f32, name=f"sumSa_{h}")
                nc.vector.tensor_reduce(sumSa, SA, mybir.AxisListType.X,
                                        OP.add)
                PR = fin.tile([128, KB, K], f32, name=f"PR_{h}", tag="PR")
                nc.vector.tensor_tensor(
                    PR,
                    Ab[:, :, None].to_broadcast([128, KB, K]),
                    R_T[:, None, :].to_broadcast([128, KB, K]),
                    OP.mult)
                SR = fin.tile([128, KB, K], f32, name=f"SR_{h}", tag="SR")
                nc.scalar.activation(SR, PR, AF.Exp, scale=1.0 / TAU)
                sumSr = fin.tile([128, KB], f32, name=f"sumSr_{h}")
                nc.vector.tensor_reduce(sumSr, SR, mybir.AxisListType.X,
                                        OP.add)

                dA = fin.tile([128, KB], f32, name=f"dA_{h}")
                nc.vector.tensor_tensor(dA, Ab, Ab, OP.mult)
                eA = fin.tile([128, KB], f32, name=f"eA_{h}")
                nc.scalar.activation(eA, dA, AF.Exp, scale=1.0 / TAU)
                dR = fin.tile([128, KB], f32, name=f"dR_{h}")
                nc.vector.tensor_tensor(dR, Ab, Rb, OP.mult)
                fp125 = fin.tile([128, KB], f32, name=f"fp125_{h}")
                nc.scalar.activation(fp125, dR, AF.Exp, scale=1.0 / TAU,
                                     bias=bias_lnc)
                sums[h] = (sumSa, sumSr, eA, fp125, dR)

            for h in range(2):
                sumSa, sumSr, eA, fp125, dR = sums[h]
                fn = fin.tile([128, KB], f32, name=f"fn_{h}")
                nc.vector.tensor_tensor(fn, sumSa, sumSr, OP.add)
                nc.vector.scalar_tensor_tensor(
                    fn, eA, -1.0, fn, OP.mult, OP.add)
                nc.vector.tensor_tensor(fn, fn, fp125, OP.add)
                lg = fin.tile([128, KB], f32, name=f"lg_{h}")
                nc.scalar.activation(lg, fn, AF.Ln, bias=bias_eps)

                # contrib_h[p] = sum_k (lg - 2*dR)
                tmp = fin.tile([128, KB], f32, name=f"tmp_{h}")
                ch = fin.tile([128, 1], f32, name=f"contrib_{h}")
                nc.vector.scalar_tensor_tensor(
                    tmp, dR, -1.0 / TAU, lg, OP.mult, OP.add, accum_out=ch)
                if h == 0:
                    nc.vector.tensor_copy(total, ch)
                else:
                    nc.vector.tensor_tensor(total, total, ch, OP.add)

            # partition-sum of this core's partial via ones matmul
            psF = psum.tile([1, 1], f32, name="psF", tag="pscalar")
            nc.tensor.matmul(psF, ones_sb, total, start=True, stop=True)
            part = fin.tile([1, 1], f32)
            nc.vector.tensor_copy(part, psF)

            # AllGather the 8 partials; every core sums them -> same scalar
            ag_in = dram.tile([1, 1], f32)
            ag_out = dram.tile([NCORES, 1], f32)
            nc.sync.dma_start(ag_in, part)
            nc.gpsimd.collective_compute(
                "AllGather", mybir.AluOpType.bypass,
                replica_groups=[list(range(NCORES))],
                ins=[ag_in.opt()], outs=[ag_out.opt()],
            )
            ag_sb = fin.tile([NCORES, 1], f32)
            nc.sync.dma_start(ag_sb, ag_out)
            psG = psum.tile([1, 1], f32, name="psG", tag="pscalar")
            nc.tensor.matmul(psG, ones_sb[0:NCORES, :], ag_sb,
                             start=True, stop=True)
            res = fin.tile([1, 1], f32)
            nc.vector.tensor_scalar_mul(res, psG, 1.0 / D)
            nc.sync.dma_start(out[:, :], res)

    nc.compile()
    return nc


def _get_nc(repeat_main=1, repeat_ar=1):
    key = ("nc", repeat_main, repeat_ar)
    if key not in _cache:
        _cache[key] = _build(repeat_main, repeat_ar)
    return _cache[key]


def _shard(arr, pad_value):
    """Split [N, ...] into NCORES shards of NPAD rows, padding the tail."""
    shards = []
    for i in range(NCORES):
        lo = min(i * NPAD, N)
        hi = min(lo + NPAD, N)
        part = arr[lo:hi]
        if part.shape[0] < NPAD:
            pad_shape = (NPAD - part.shape[0],) + arr.shape[1:]
            part = np.concatenate(
                [part, np.full(pad_shape, pad_value, dtype=arr.dtype)])
        shards.append(np.ascontiguousarray(part))
    return shards


def _shard_feat(arr):
    """[N, D] f32 -> NCORES shards of [NPAD, D] rows (zero-padded tail)."""
    return _shard(arr, 0.0)


def _ksel(core):
    sel = np.zeros((128, 2 * KB), dtype=np.float32)
    for i in range(KB):
        sel[core * KB + i, i] = 1.0            # A rows live at 0:64
        sel[64 + core * KB + i, KB + i] = 1.0  # R rows live at 64:128
    return sel


def _in_maps(inputs):
    fa_s = _shard_feat(np.asarray(inputs["atac_feature"], dtype=np.float32))
    fr_s = _shard_feat(np.asarray(inputs["rna_feature"], dtype=np.float32))
    la_s = _shard(np.asarray(inputs["atac_label"], dtype=np.int32), -1)
    lr_s = _shard(np.asarray(inputs["rna_label"], dtype=np.int32), -1)
    return [
        {"fa": fa_s[i], "fr": fr_s[i], "la": la_s[i], "lr": lr_s[i],
         "ksel": _ksel(i)}
        for i in range(NCORES)
    ]


def run_with_results(atac_feature, rna_feature, atac_label, rna_label,
                     **run_kwargs):
    from concourse import bass_utils

    nc = _get_nc()
    in_maps = _in_maps({
        "atac_feature": atac_feature, "rna_feature": rna_feature,
        "atac_label": atac_label, "rna_label": rna_label})
    return bass_utils.run_bass_kernel_spmd(
        nc, in_maps, core_ids=list(range(NCORES)), **run_kwargs)


def kernel(atac_feature, rna_feature, atac_label, rna_label):
    res = run_with_results(atac_feature, rna_feature, atac_label, rna_label)
    return np.asarray(
        res.results[0]["out"], dtype=np.float32).reshape(())



# revision 14
# speedup vs baseline: 20.0353x; 20.0353x over previous
"""Trainium2 Bass kernel for nn_ContrastiveLoss (prototype InfoNCE loss).

Strategy (data-parallel over the N=100k cell axis, 8 NeuronCores):
  - Each core gets a 12544-row shard (rows padded with label=-1 / feat=0),
    laid out so partition p owns the contiguous rows [p*98, (p+1)*98):
    every feature DMA line is one contiguous run (16 KB per partition per
    chunk), 128 descriptors per transfer.  The row permutation is free —
    segment-sum is permutation invariant and labels follow the same map.
  - Labels load directly as [128, 98] (i32->f32 cast DMA, no transpose).
  - Per 128-row tile, a one-hot [128,64] matrix is built on-chip (DVE
    is_equal against an iota constant) and a bf16 matmul one_hot.T @ feat
    accumulates per-class sums into PSUM (fp32).  Features are cast
    f32->bf16 in-flight by the SWDGE DMA; counts are never materialized
    (they cancel inside l2-normalize).
  - One 8-core AllReduce of the [128, 256] packed (atac|rna) sums.
  - The tiny K=64 InfoNCE is replicated on every core (no second
    collective): normalize via folded PE transpose, bf16 broadcast
    products, ACT exp, free-axis reductions; final partition-sum via a
    ones matmul.  Every core writes the same scalar loss.
  - Validated rel err vs the f32 reference: 1.5e-5.
"""
import sys

sys.path.insert(0, "/opt/trn_rl_repo")

import math
import numpy as np
from contextlib import ExitStack

N, D, K = 100000, 256, 64
NCORES = 8
NTILES = 98               # tiles of 128 rows per core
NPAD = NTILES * 128       # 12544 rows per core (total 100352 >= 100000)
CH = 16                   # 128-row tiles per DMA chunk (2 MB f32 reads)
NCHUNKS = (NTILES + CH - 1) // CH   # 12 full chunks + 1 chunk of 2
TAU = 0.5
EPS = 1e-8
C_FP = 2 * K - 3          # coefficient of Fp in Fn:  sum_{j!=k}(Sa+Sr)+2(K-1)Fp
                          #   = rowsum(Sa)+rowsum(Sr) - exp(2A.A) + (2K-3)*Fp
KB = K // NCORES          # k-rows of the InfoNCE computed per core

_cache = {}


def _build(repeat_main=1, repeat_ar=1, repeat_all=1, serial_reps=0,
           ch=CH, no_ar=False):
    import concourse.bacc as bacc
    import concourse.tile as tile
    from concourse import mybir

    f32, bf16, i32 = mybir.dt.float32, mybir.dt.bfloat16, mybir.dt.int32
    AF = mybir.ActivationFunctionType
    OP = mybir.AluOpType

    nc = bacc.Bacc(None, target_bir_lowering=False, debug=False,
                   num_devices=NCORES)

    fa = nc.dram_tensor("fa", [NPAD, D], f32, kind="ExternalInput")
    fr = nc.dram_tensor("fr", [NPAD, D], f32, kind="ExternalInput")
    la = nc.dram_tensor("la", [NPAD], i32, kind="ExternalInput")
    lr = nc.dram_tensor("lr", [NPAD], i32, kind="ExternalInput")
    out = nc.dram_tensor("out", [1, 1], f32, kind="ExternalOutput")

    iota_c = nc.inline_tensor(
        np.tile(np.arange(K, dtype=np.float32), (128, 1)), name="iota_c")
    ident_c = nc.inline_tensor(np.eye(128, dtype=np.float32), name="ident_c")
    ones_c = nc.inline_tensor(np.ones((128, 1), dtype=np.float32),
                              name="ones_c")

    with tile.TileContext(nc) as tc, ExitStack() as ctx:
        consts = ctx.enter_context(tc.tile_pool(name="consts", bufs=1))
        psum = ctx.enter_context(tc.tile_pool(name="psum", bufs=1,
                                              space="PSUM"))
        dram = ctx.enter_context(tc.tile_pool(name="dram", bufs=1,
                                              space="DRAM"))

        iota_sb = consts.tile([128, K], f32)
        nc.sync.dma_start(iota_sb, iota_c[:, :])
        warm = consts.tile([1, 1], f32)
        nc.vector.memset(warm, 1.0)
        nc.scalar.activation(warm, warm, AF.Exp)
        nc.scalar.activation(warm, warm, AF.Ln)
        ident_sb = consts.tile([128, 128], f32)
        nc.sync.dma_start(ident_sb, ident_c[:, :])
        ones_sb = consts.tile([128, 1], f32)
        nc.sync.dma_start(ones_sb, ones_c[:, :])

        gate = None
        fa_src, fr_src = fa, fr
        reps = repeat_all
        if serial_reps and serial_reps > 1:
            # Timing-only mode: features are read through DRAM scratch, and
            # each rep's tail writes into the first row of every chunk, so
            # rep r+1 has a true RAW dependency on rep r's completion. The
            # slope over serial_reps is the serial per-exec span.  (The
            # gate writes corrupt a few scratch floats - output is garbage,
            # which is fine for timing builds.)
            sa = dram.tile([NPAD, D], f32, name="sa")
            sr = dram.tile([NPAD, D], f32, name="sr")
            nc.sync.dma_start(sa, fa[:, :])
            nc.sync.dma_start(sr, fr[:, :])
            fa_src, fr_src = sa, sr
            gate = (sa, sr)
            reps = serial_reps

        for _rep_all in range(reps):
            _emit_body(nc, tc, tile, mybir, psum, dram, consts,
                       fa_src, fr_src, la, lr, out,
                       iota_sb, ident_sb, ones_sb,
                       repeat_main, repeat_ar, gate, ch, no_ar)

    nc.compile()
    return nc


def _emit_body(nc, tc, tile, mybir, psum, dram, consts,
               fa, fr, la, lr, out,
               iota_sb, ident_sb, ones_sb,
               repeat_main, repeat_ar, gate=None, chsz=CH, no_ar=False):
    f32, bf16, i32 = mybir.dt.float32, mybir.dt.bfloat16, mybir.dt.int32
    AF = mybir.ActivationFunctionType
    OP = mybir.AluOpType
    nchunks = (NTILES + chsz - 1) // chsz

    if True:
        # ---------------- main phase: segment sums + counts ----------------
        with tc.tile_pool(name="labels", bufs=1) as labels, \
             tc.tile_pool(name="oh", bufs=1) as ohp, \
             tc.tile_pool(name="feat", bufs=3) as featp:

            # label prep: [NPAD] i32 -> (cast DMA) [128, 98] f32 directly.
            # Row r of the shard lives at (partition r//98, slot r%98):
            # per-partition lines are contiguous in DRAM, and the matching
            # feature layout below gives contiguous 16 KB DMA lines.
            labT = {}
            for nm, lab in (("a", la), ("r", lr)):
                lt = labels.tile([128, NTILES], f32, name=f"labT_{nm}")
                nc.gpsimd.dma_start(
                    lt, lab[:].rearrange("(p j) -> p j", j=NTILES))
                labT[nm] = lt

            # one-hots for all tiles: oh[p, t, k] = (label[t*128+p] == k)
            # split [0:CH] / [CH:] so the first chunk's matmuls start early
            oh = {}
            for nm in ("a", "r"):
                o = ohp.tile([128, NTILES, K], bf16, name=f"oh_{nm}")
                for lo, hi in ((0, chsz), (chsz, NTILES)):
                    w = hi - lo
                    nc.vector.tensor_tensor(
                        o[:, lo:hi, :],
                        iota_sb[:, None, :].to_broadcast([128, w, K]),
                        labT[nm][:, lo:hi, None].to_broadcast([128, w, K]),
                        OP.is_equal,
                    )
                oh[nm] = o

            # Full-partition PSUM tiles so each accumulator owns its bank
            # at base_partition 0 (packing two [64,*] tiles into one bank
            # makes the second chain a col-tiled matmul, which corrupts
            # interleaved accumulation -- seen on HW).
            psA_full = psum.tile([128, D], f32)
            psR_full = psum.tile([128, D], f32)
            psA = psA_full[0:K, :]
            psR = psR_full[0:K, :]

            for rep in range(repeat_main):
                for c in range(nchunks):
                    w = min(chsz, NTILES - c * chsz)
                    r0 = c * chsz * 128
                    fts = {}
                    for nm, feat in (("a", fa), ("r", fr)):
                        ft = featp.tile([128, chsz, D], bf16, name=f"ft_{nm}",
                                        tag=f"ft_{nm}")
                        nc.gpsimd.dma_start(
                            ft[:, :w, :],
                            feat[:, :].rearrange(
                                "(p j) e -> p j e", j=NTILES)[:, c * chsz:c * chsz + w, :],
                        )
                        fts[nm] = ft
                    for j in range(w):
                        t = c * chsz + j
                        nc.tensor.matmul(psA, oh["a"][:, t, :],
                                         fts["a"][:, j, :],
                                         start=(t == 0),
                                         stop=(t == NTILES - 1))
                        nc.tensor.matmul(psR, oh["r"][:, t, :],
                                         fts["r"][:, j, :],
                                         start=(t == 0),
                                         stop=(t == NTILES - 1))

            comb = consts.tile([128, D], f32)
            nc.vector.tensor_copy(comb[0:K, :], psA)
            nc.vector.tensor_copy(comb[K:128, :], psR)

        # ---------------- AllReduce sums+counts across the 8 cores ---------
        d_in = dram.tile([128, D], f32)
        d_out = dram.tile([128, D], f32)
        nc.sync.dma_start(d_in, comb)
        if no_ar:
            # timing-only: skip the collective (output wrong by 8x)
            nc.sync.dma_start(d_out, d_in)
        else:
            for _rep in range(repeat_ar):
                nc.gpsimd.collective_compute(
                    "AllReduce", mybir.AluOpType.add,
                    replica_groups=[list(range(NCORES))],
                    ins=[d_in.opt()], outs=[d_out.opt()],
                )

        # ---- tiny K x K x D InfoNCE, fully replicated on every core ------
        # Each core computes all 64 rows (bf16 products, 2x DVE rate) and
        # writes the same scalar; no second collective needed.
        with tc.tile_pool(name="fin", bufs=1) as fin, \
             tc.tile_pool(name="pst", bufs=1, space="PSUM") as pstp:
            allr = fin.tile([128, D], f32)
            nc.sync.dma_start(allr, d_out)

            # l2norm(sums/counts) == sums/||sums||: counts cancel, so we
            # never materialize them.  rinv = exp(-0.5*ln(sum(s^2)))
            sq = fin.tile([128, D], f32)
            ss = fin.tile([128, 1], f32)
            nc.scalar.activation(sq, allr, AF.Square, accum_out=ss)
            lnss = fin.tile([128, 1], f32)
            nc.scalar.activation(lnss, ss, AF.Ln)
            rinv = fin.tile([128, 1], f32)
            nc.scalar.activation(rinv, lnss, AF.Exp, scale=-0.5)

            # Fold normalization into the PE transpose:
            #   pT_h[d, i] = sum_p allr[p, h*128+d] * (ident*rinv)[p, i]
            #              = allr[i, h*128+d] * rinv[i]  (normalized, transposed)
            dscale = fin.tile([128, 128], f32)
            nc.vector.tensor_scalar_mul(dscale, ident_sb, rinv)

            bias_lnc = fin.tile([128, 1], f32)
            nc.vector.memset(bias_lnc, math.log(C_FP))
            bias_eps = fin.tile([128, 1], f32)
            nc.vector.memset(bias_eps, EPS)

            total = fin.tile([128, 1], f32)
            for h in range(2):
                half = allr[:, h * 128:(h + 1) * 128]
                pst = pstp.tile([128, 128], f32, name=f"pst_{h}", tag="pst")
                nc.tensor.matmul(pst, half, dscale, start=True, stop=True)
                pT16 = fin.tile([128, 128], bf16, name=f"pT16_{h}")
                nc.vector.tensor_copy(pT16, pst)
                A16 = pT16[:, 0:K]          # [128, 64] normalized A rows
                R16 = pT16[:, K:128]        # [128, 64] normalized R rows

                PA = fin.tile([128, K, K], bf16, name=f"PA_{h}", tag="PA")
                nc.vector.tensor_tensor(
                    PA,
                    A16[:, :, None].to_broadcast([128, K, K]),
                    A16[:, None, :].to_broadcast([128, K, K]),
                    OP.mult)
                SA = fin.tile([128, K, K], bf16, name=f"SA_{h}", tag="SA")
                nc.scalar.activation(SA, PA, AF.Exp, scale=1.0 / TAU)
                sumSa = fin.tile([128, K], f32, name=f"sumSa_{h}")
                nc.vector.tensor_reduce(sumSa, SA, mybir.AxisListType.X,
                                        OP.add)
                PR = fin.tile([128, K, K], bf16, name=f"PR_{h}", tag="PR")
                nc.vector.tensor_tensor(
                    PR,
                    A16[:, :, None].to_broadcast([128, K, K]),
                    R16[:, None, :].to_broadcast([128, K, K]),
                    OP.mult)
                SR = fin.tile([128, K, K], bf16, name=f"SR_{h}", tag="SR")
                nc.scalar.activation(SR, PR, AF.Exp, scale=1.0 / TAU)
                sumSr = fin.tile([128, K], f32, name=f"sumSr_{h}")
                nc.vector.tensor_reduce(sumSr, SR, mybir.AxisListType.X,
                                        OP.add)

                dA = fin.tile([128, K], f32, name=f"dA_{h}")
                nc.vector.tensor_tensor(dA, A16, A16, OP.mult)
                eA = fin.tile([128, K], f32, name=f"eA_{h}")
                nc.scalar.activation(eA, dA, AF.Exp, scale=1.0 / TAU)
                dR = fin.tile([128, K], f32, name=f"dR_{h}")
                nc.vector.tensor_tensor(dR, A16, R16, OP.mult)
                fp125 = fin.tile([128, K], f32, name=f"fp125_{h}")
                nc.scalar.activation(fp125, dR, AF.Exp, scale=1.0 / TAU,
                                     bias=bias_lnc)

                fn = fin.tile([128, K], f32, name=f"fn_{h}")
                nc.vector.tensor_tensor(fn, sumSa, sumSr, OP.add)
                nc.vector.scalar_tensor_tensor(
                    fn, eA, -1.0, fn, OP.mult, OP.add)
                nc.vector.tensor_tensor(fn, fn, fp125, OP.add)
                lg = fin.tile([128, K], f32, name=f"lg_{h}")
                nc.scalar.activation(lg, fn, AF.Ln, bias=bias_eps)

                # contrib_h[p] = sum_k (lg - 2*dR)
                tmp = fin.tile([128, K], f32, name=f"tmp_{h}")
                ch = fin.tile([128, 1], f32, name=f"contrib_{h}")
                nc.vector.scalar_tensor_tensor(
                    tmp, dR, -1.0 / TAU, lg, OP.mult, OP.add, accum_out=ch)
                if h == 0:
                    nc.vector.tensor_copy(total, ch)
                else:
                    nc.vector.tensor_tensor(total, total, ch, OP.add)

            # loss = (sum over d partitions of total) / D  (identical on
            # every core -- sum over both halves' partitions is d=0..255)
            psF = psum.tile([1, 1], f32, name="psF", tag="pscalar")
            nc.tensor.matmul(psF, ones_sb, total, start=True, stop=True)
            res = fin.tile([1, 1], f32)
            nc.vector.tensor_scalar_mul(res, psF, 1.0 / D)
            nc.sync.dma_start(out[:, :], res)

            if gate is not None:
                # serial-reps timing mode: stamp the first row of every
                # chunk of both feature scratches so the next rep's DMAs
                # RAW-depend on this rep's tail.
                ga, gr = gate
                nc.sync.dma_start(ga[0:nchunks * chsz:chsz, 0:1],
                                  total[0:nchunks, 0:1])
                nc.sync.dma_start(gr[0:nchunks * chsz:chsz, 0:1],
                                  total[0:nchunks, 0:1])


def _get_nc(repeat_main=1, repeat_ar=1, repeat_all=1, serial_reps=0,
            ch=CH, no_ar=False):
    key = ("nc", repeat_main, repeat_ar, repeat_all, serial_reps, ch, no_ar)
    if key not in _cache:
        _cache[key] = _build(repeat_main, repeat_ar, repeat_all, serial_reps,
                             ch, no_ar)
    return _cache[key]


def _shard(arr, pad_value):
    """Split [N, ...] into NCORES shards of NPAD rows, padding the tail."""
    shards = []
    for i in range(NCORES):
        lo = min(i * NPAD, N)
        hi = min(lo + NPAD, N)
        part = arr[lo:hi]
        if part.shape[0] < NPAD:
            pad_shape = (NPAD - part.shape[0],) + arr.shape[1:]
            part = np.concatenate(
                [part, np.full(pad_shape, pad_value, dtype=arr.dtype)])
        shards.append(np.ascontiguousarray(part))
    return shards


def _shard_feat(arr):
    """[N, D] f32 -> NCORES shards of [NPAD, D] rows (zero-padded tail)."""
    return _shard(arr, 0.0)


def _in_maps(inputs):
    fa_s = _shard_feat(np.asarray(inputs["atac_feature"], dtype=np.float32))
    fr_s = _shard_feat(np.asarray(inputs["rna_feature"], dtype=np.float32))
    la_s = _shard(np.asarray(inputs["atac_label"], dtype=np.int32), -1)
    lr_s = _shard(np.asarray(inputs["rna_label"], dtype=np.int32), -1)
    return [
        {"fa": fa_s[i], "fr": fr_s[i], "la": la_s[i], "lr": lr_s[i]}
        for i in range(NCORES)
    ]


def run_with_results(atac_feature, rna_feature, atac_label, rna_label,
                     **run_kwargs):
    from concourse import bass_utils

    nc = _get_nc()
    in_maps = _in_maps({
        "atac_feature": atac_feature, "rna_feature": rna_feature,
        "atac_label": atac_label, "rna_label": rna_label})
    return bass_utils.run_bass_kernel_spmd(
        nc, in_maps, core_ids=list(range(NCORES)), **run_kwargs)


def kernel(atac_feature, rna_feature, atac_label, rna_label):
    res = run_with_results(atac_feature, rna_feature, atac_label, rna_label)
    return np.asarray(
        res.results[0]["out"], dtype=np.float32).reshape(())



# revision 16
# speedup vs baseline: 29.3724x; 1.4660x over previous
"""Trainium2 Bass kernel for nn_ContrastiveLoss (prototype InfoNCE loss).

Strategy (data-parallel over the N=100k cell axis, 8 NeuronCores):
  - Each core gets a 12544-row shard (rows padded with label=-1 / feat=0),
    laid out so partition p owns the contiguous rows [p*98, (p+1)*98):
    every feature DMA line is one contiguous run (16 KB per partition per
    chunk), 128 descriptors per transfer.  The row permutation is free —
    segment-sum is permutation invariant and labels follow the same map.
  - Labels load directly as [128, 98] (i32->f32 cast DMA, no transpose).
  - Per 128-row tile, a one-hot [128,64] matrix is built on-chip (DVE
    is_equal against an iota constant) and a bf16 matmul one_hot.T @ feat
    accumulates per-class sums into PSUM (fp32).  Features are cast
    f32->bf16 in-flight by the SWDGE DMA; counts are never materialized
    (they cancel inside l2-normalize).
  - One 8-core AllReduce of the [128, 256] packed (atac|rna) sums.
  - The tiny K=64 InfoNCE is replicated on every core (no second
    collective): normalize via folded PE transpose, bf16 broadcast
    products, ACT exp, free-axis reductions; final partition-sum via a
    ones matmul.  Every core writes the same scalar loss.
  - Validated rel err vs the f32 reference: 1.5e-5.
"""
import sys

sys.path.insert(0, "/opt/trn_rl_repo")

import math
import numpy as np
from contextlib import ExitStack

N, D, K = 100000, 256, 64
NCORES = 8
NTILES = 98               # tiles of 128 rows per core
NPAD = NTILES * 128       # 12544 rows per core (total 100352 >= 100000)
CH = 16                   # 128-row tiles per DMA chunk (2 MB f32 reads)
NCHUNKS = (NTILES + CH - 1) // CH   # 12 full chunks + 1 chunk of 2
TAU = 0.5
EPS = 1e-8
C_FP = 2 * K - 3          # coefficient of Fp in Fn:  sum_{j!=k}(Sa+Sr)+2(K-1)Fp
                          #   = rowsum(Sa)+rowsum(Sr) - exp(2A.A) + (2K-3)*Fp
KB = K // NCORES          # k-rows of the InfoNCE computed per core

_cache = {}


def _build(repeat_main=1, repeat_ar=1, repeat_all=1, serial_reps=0,
           ch=CH, no_ar=False):
    import concourse.bacc as bacc
    import concourse.tile as tile
    from concourse import mybir

    f32, bf16, i32 = mybir.dt.float32, mybir.dt.bfloat16, mybir.dt.int32
    AF = mybir.ActivationFunctionType
    OP = mybir.AluOpType

    nc = bacc.Bacc(None, target_bir_lowering=False, debug=False,
                   num_devices=NCORES)

    fa = nc.dram_tensor("fa", [NPAD, D], f32, kind="ExternalInput")
    fr = nc.dram_tensor("fr", [NPAD, D], f32, kind="ExternalInput")
    la = nc.dram_tensor("la", [NPAD], i32, kind="ExternalInput")
    lr = nc.dram_tensor("lr", [NPAD], i32, kind="ExternalInput")
    out = nc.dram_tensor("out", [1, 1], f32, kind="ExternalOutput")

    iota_c = nc.inline_tensor(
        np.tile(np.arange(K, dtype=np.float32), (128, 1)), name="iota_c")
    ident_c = nc.inline_tensor(np.eye(128, dtype=np.float32), name="ident_c")
    ones_c = nc.inline_tensor(np.ones((128, 1), dtype=np.float32),
                              name="ones_c")

    with tile.TileContext(nc) as tc, ExitStack() as ctx:
        consts = ctx.enter_context(tc.tile_pool(name="consts", bufs=1))
        psum = ctx.enter_context(tc.tile_pool(name="psum", bufs=1,
                                              space="PSUM"))
        dram = ctx.enter_context(tc.tile_pool(name="dram", bufs=1,
                                              space="DRAM"))

        iota_sb = consts.tile([128, K], f32)
        nc.sync.dma_start(iota_sb, iota_c[:, :])
        warm = consts.tile([1, 1], f32)
        nc.vector.memset(warm, 1.0)
        nc.scalar.activation(warm, warm, AF.Exp)
        nc.scalar.activation(warm, warm, AF.Ln)
        ident_sb = consts.tile([128, 128], f32)
        nc.sync.dma_start(ident_sb, ident_c[:, :])
        ones_sb = consts.tile([128, 1], f32)
        nc.sync.dma_start(ones_sb, ones_c[:, :])

        gate = None
        fa_src, fr_src = fa, fr
        reps = repeat_all
        if serial_reps and serial_reps > 1:
            # Timing-only mode: features are read through DRAM scratch, and
            # each rep's tail writes into the first row of every chunk, so
            # rep r+1 has a true RAW dependency on rep r's completion. The
            # slope over serial_reps is the serial per-exec span.  (The
            # gate writes corrupt a few scratch floats - output is garbage,
            # which is fine for timing builds.)
            sa = dram.tile([NPAD, D], f32, name="sa")
            sr = dram.tile([NPAD, D], f32, name="sr")
            nc.sync.dma_start(sa, fa[:, :])
            nc.sync.dma_start(sr, fr[:, :])
            fa_src, fr_src = sa, sr
            gate = (sa, sr)
            reps = serial_reps

        for _rep_all in range(reps):
            _emit_body(nc, tc, tile, mybir, psum, dram, consts,
                       fa_src, fr_src, la, lr, out,
                       iota_sb, ident_sb, ones_sb,
                       repeat_main, repeat_ar, gate, ch, no_ar)

    nc.compile()
    return nc


def _emit_body(nc, tc, tile, mybir, psum, dram, consts,
               fa, fr, la, lr, out,
               iota_sb, ident_sb, ones_sb,
               repeat_main, repeat_ar, gate=None, chsz=CH, no_ar=False):
    f32, bf16, i32 = mybir.dt.float32, mybir.dt.bfloat16, mybir.dt.int32
    AF = mybir.ActivationFunctionType
    OP = mybir.AluOpType
    nchunks = (NTILES + chsz - 1) // chsz

    if True:
        # ---------------- main phase: segment sums + counts ----------------
        with tc.tile_pool(name="labels", bufs=1) as labels, \
             tc.tile_pool(name="oh", bufs=1) as ohp, \
             tc.tile_pool(name="feat", bufs=3) as featp:

            # label prep: [NPAD] i32 -> (cast DMA) [128, 98] f32 directly.
            # Row r of the shard lives at (partition r//98, slot r%98):
            # per-partition lines are contiguous in DRAM, and the matching
            # feature layout below gives contiguous 16 KB DMA lines.
            labT = {}
            for nm, lab in (("a", la), ("r", lr)):
                lt = labels.tile([128, NTILES], f32, name=f"labT_{nm}")
                nc.gpsimd.dma_start(
                    lt, lab[:].rearrange("(p j) -> p j", j=NTILES))
                labT[nm] = lt

            # one-hots for all tiles: oh[p, t, k] = (label[t*128+p] == k)
            # split [0:CH] / [CH:] so the first chunk's matmuls start early
            oh = {}
            for nm in ("a", "r"):
                o = ohp.tile([128, NTILES, K], bf16, name=f"oh_{nm}")
                for lo, hi in ((0, chsz), (chsz, NTILES)):
                    w = hi - lo
                    nc.vector.tensor_tensor(
                        o[:, lo:hi, :],
                        iota_sb[:, None, :].to_broadcast([128, w, K]),
                        labT[nm][:, lo:hi, None].to_broadcast([128, w, K]),
                        OP.is_equal,
                    )
                oh[nm] = o

            # Full-partition PSUM tiles so each accumulator owns its bank
            # at base_partition 0 (packing two [64,*] tiles into one bank
            # makes the second chain a col-tiled matmul, which corrupts
            # interleaved accumulation -- seen on HW).
            psA_full = psum.tile([128, D], f32)
            psR_full = psum.tile([128, D], f32)
            psA = psA_full[0:K, :]
            psR = psR_full[0:K, :]

            for rep in range(repeat_main):
                for c in range(nchunks):
                    w = min(chsz, NTILES - c * chsz)
                    r0 = c * chsz * 128
                    fts = {}
                    for nm, feat in (("a", fa), ("r", fr)):
                        ft = featp.tile([128, chsz, D], bf16, name=f"ft_{nm}",
                                        tag=f"ft_{nm}")
                        nc.gpsimd.dma_start(
                            ft[:, :w, :],
                            feat[:, :].rearrange(
                                "(p j) e -> p j e", j=NTILES)[:, c * chsz:c * chsz + w, :],
                        )
                        fts[nm] = ft
                    for j in range(w):
                        t = c * chsz + j
                        nc.tensor.matmul(psA, oh["a"][:, t, :],
                                         fts["a"][:, j, :],
                                         start=(t == 0),
                                         stop=(t == NTILES - 1))
                        nc.tensor.matmul(psR, oh["r"][:, t, :],
                                         fts["r"][:, j, :],
                                         start=(t == 0),
                                         stop=(t == NTILES - 1))

            comb = consts.tile([128, D], f32)
            nc.vector.tensor_copy(comb[0:K, :], psA)
            nc.vector.tensor_copy(comb[K:128, :], psR)

        # ---------------- AllReduce sums+counts across the 8 cores ---------
        d_in = dram.tile([128, D], f32)
        d_out = dram.tile([128, D], f32)
        nc.sync.dma_start(d_in, comb)
        if no_ar:
            # timing-only: skip the collective (output wrong by 8x)
            nc.sync.dma_start(d_out, d_in)
        else:
            for _rep in range(repeat_ar):
                nc.gpsimd.collective_compute(
                    "AllReduce", mybir.AluOpType.add,
                    replica_groups=[list(range(NCORES))],
                    ins=[d_in.opt()], outs=[d_out.opt()],
                )

        # ---- tiny K x K x D InfoNCE, fully replicated on every core ------
        # Each core computes all 64 rows (bf16 products, 2x DVE rate) and
        # writes the same scalar; no second collective needed.
        with tc.tile_pool(name="fin", bufs=1) as fin, \
             tc.tile_pool(name="pst", bufs=1, space="PSUM") as pstp:
            allr = fin.tile([128, D], f32)
            nc.sync.dma_start(allr, d_out)

            # l2norm(sums/counts) == sums/||sums||: counts cancel, so we
            # never materialize them.  rinv = exp(-0.5*ln(sum(s^2)))
            sq = fin.tile([128, D], f32)
            ss = fin.tile([128, 1], f32)
            nc.scalar.activation(sq, allr, AF.Square, accum_out=ss)
            lnss = fin.tile([128, 1], f32)
            nc.scalar.activation(lnss, ss, AF.Ln)
            rinv = fin.tile([128, 1], f32)
            nc.scalar.activation(rinv, lnss, AF.Exp, scale=-0.5)

            # Fold normalization into the PE transpose:
            #   pT_h[d, i] = sum_p allr[p, h*128+d] * (ident*rinv)[p, i]
            #              = allr[i, h*128+d] * rinv[i]  (normalized, transposed)
            dscale = fin.tile([128, 128], f32)
            nc.vector.tensor_scalar_mul(dscale, ident_sb, rinv)

            bias_lnc = fin.tile([128, 1], f32)
            nc.vector.memset(bias_lnc, math.log(C_FP))
            bias_eps = fin.tile([128, 1], f32)
            nc.vector.memset(bias_eps, EPS)

            total = fin.tile([128, 1], f32)
            # pass 1: products + all ACT Exps (one exp-table load);
            # pass 2: Fn combines + ACT Lns (one ln-table load)
            sums = {}
            for h in range(2):
                half = allr[:, h * 128:(h + 1) * 128]
                pst = pstp.tile([128, 128], f32, name=f"pst_{h}", tag="pst")
                nc.tensor.matmul(pst, half, dscale, start=True, stop=True)
                pT16 = fin.tile([128, 128], bf16, name=f"pT16_{h}")
                nc.vector.tensor_copy(pT16, pst)
                A16 = pT16[:, 0:K]          # [128, 64] normalized A rows
                R16 = pT16[:, K:128]        # [128, 64] normalized R rows

                PA = fin.tile([128, K, K], bf16, name=f"PA_{h}", tag="PA")
                nc.vector.tensor_tensor(
                    PA,
                    A16[:, :, None].to_broadcast([128, K, K]),
                    A16[:, None, :].to_broadcast([128, K, K]),
                    OP.mult)
                SA = fin.tile([128, K, K], bf16, name=f"SA_{h}", tag="SA")
                nc.scalar.activation(SA, PA, AF.Exp, scale=1.0 / TAU)
                sumSa = fin.tile([128, K], f32, name=f"sumSa_{h}")
                nc.vector.tensor_reduce(sumSa, SA, mybir.AxisListType.X,
                                        OP.add)
                PR = fin.tile([128, K, K], bf16, name=f"PR_{h}", tag="PR")
                nc.vector.tensor_tensor(
                    PR,
                    A16[:, :, None].to_broadcast([128, K, K]),
                    R16[:, None, :].to_broadcast([128, K, K]),
                    OP.mult)
                SR = fin.tile([128, K, K], bf16, name=f"SR_{h}", tag="SR")
                nc.scalar.activation(SR, PR, AF.Exp, scale=1.0 / TAU)
                sumSr = fin.tile([128, K], f32, name=f"sumSr_{h}")
                nc.vector.tensor_reduce(sumSr, SR, mybir.AxisListType.X,
                                        OP.add)

                dA = fin.tile([128, K], f32, name=f"dA_{h}")
                nc.vector.tensor_tensor(dA, A16, A16, OP.mult)
                eA = fin.tile([128, K], f32, name=f"eA_{h}")
                nc.scalar.activation(eA, dA, AF.Exp, scale=1.0 / TAU)
                dR = fin.tile([128, K], f32, name=f"dR_{h}")
                nc.vector.tensor_tensor(dR, A16, R16, OP.mult)
                fp125 = fin.tile([128, K], f32, name=f"fp125_{h}")
                nc.scalar.activation(fp125, dR, AF.Exp, scale=1.0 / TAU,
                                     bias=bias_lnc)
                sums[h] = (sumSa, sumSr, eA, fp125, dR)

            for h in range(2):
                sumSa, sumSr, eA, fp125, dR = sums[h]
                fn = fin.tile([128, K], f32, name=f"fn_{h}")
                nc.vector.tensor_tensor(fn, sumSa, sumSr, OP.add)
                nc.vector.scalar_tensor_tensor(
                    fn, eA, -1.0, fn, OP.mult, OP.add)
                nc.vector.tensor_tensor(fn, fn, fp125, OP.add)
                lg = fin.tile([128, K], f32, name=f"lg_{h}")
                nc.scalar.activation(lg, fn, AF.Ln, bias=bias_eps)

                # contrib_h[p] = sum_k (lg - 2*dR)
                tmp = fin.tile([128, K], f32, name=f"tmp_{h}")
                ch = fin.tile([128, 1], f32, name=f"contrib_{h}")
                nc.vector.scalar_tensor_tensor(
                    tmp, dR, -1.0 / TAU, lg, OP.mult, OP.add, accum_out=ch)
                if h == 0:
                    nc.vector.tensor_copy(total, ch)
                else:
                    nc.vector.tensor_tensor(total, total, ch, OP.add)

            # loss = (sum over d partitions of total) / D  (identical on
            # every core -- sum over both halves' partitions is d=0..255)
            psF = psum.tile([1, 1], f32, name="psF", tag="pscalar")
            nc.tensor.matmul(psF, ones_sb, total, start=True, stop=True)
            res = fin.tile([1, 1], f32)
            nc.vector.tensor_scalar_mul(res, psF, 1.0 / D)
            nc.sync.dma_start(out[:, :], res)

            if gate is not None:
                # serial-reps timing mode: stamp the first row of every
                # chunk of both feature scratches so the next rep's DMAs
                # RAW-depend on this rep's tail.
                ga, gr = gate
                nc.sync.dma_start(ga[0:nchunks * chsz:chsz, 0:1],
                                  total[0:nchunks, 0:1])
                nc.sync.dma_start(gr[0:nchunks * chsz:chsz, 0:1],
                                  total[0:nchunks, 0:1])


def _get_nc(repeat_main=1, repeat_ar=1, repeat_all=1, serial_reps=0,
            ch=CH, no_ar=False):
    key = ("nc", repeat_main, repeat_ar, repeat_all, serial_reps, ch, no_ar)
    if key not in _cache:
        _cache[key] = _build(repeat_main, repeat_ar, repeat_all, serial_reps,
                             ch, no_ar)
    return _cache[key]


def _shard(arr, pad_value):
    """Split [N, ...] into NCORES shards of NPAD rows, padding the tail."""
    shards = []
    for i in range(NCORES):
        lo = min(i * NPAD, N)
        hi = min(lo + NPAD, N)
        part = arr[lo:hi]
        if part.shape[0] < NPAD:
            pad_shape = (NPAD - part.shape[0],) + arr.shape[1:]
            part = np.concatenate(
                [part, np.full(pad_shape, pad_value, dtype=arr.dtype)])
        shards.append(np.ascontiguousarray(part))
    return shards


def _shard_feat(arr):
    """[N, D] f32 -> NCORES shards of [NPAD, D] rows (zero-padded tail)."""
    return _shard(arr, 0.0)


def _in_maps(inputs):
    fa_s = _shard_feat(np.asarray(inputs["atac_feature"], dtype=np.float32))
    fr_s = _shard_feat(np.asarray(inputs["rna_feature"], dtype=np.float32))
    la_s = _shard(np.asarray(inputs["atac_label"], dtype=np.int32), -1)
    lr_s = _shard(np.asarray(inputs["rna_label"], dtype=np.int32), -1)
    return [
        {"fa": fa_s[i], "fr": fr_s[i], "la": la_s[i], "lr": lr_s[i]}
        for i in range(NCORES)
    ]


def run_with_results(atac_feature, rna_feature, atac_label, rna_label,
                     **run_kwargs):
    from concourse import bass_utils

    nc = _get_nc()
    in_maps = _in_maps({
        "atac_feature": atac_feature, "rna_feature": rna_feature,
        "atac_label": atac_label, "rna_label": rna_label})
    return bass_utils.run_bass_kernel_spmd(
        nc, in_maps, core_ids=list(range(NCORES)), **run_kwargs)


def kernel(atac_feature, rna_feature, atac_label, rna_label):
    res = run_with_results(atac_feature, rna_feature, atac_label, rna_label)
    return np.asarray(
        res.results[0]["out"], dtype=np.float32).reshape(())

